# revision 6
# baseline (speedup 1.0000x reference)
"""Trainium2 Bass kernel for nn_AttentionLayer (B=8, S=2048, D=256, U=128).

Data-parallel over the batch dim: one batch element per NeuronCore, weights
replicated. Per-core flash-attention-style layer, transpose-free layout:

  X^T  = transpose(X) via TensorE                       [D, S]
  Q^T  = W_q^T @ X^T, K^T likewise                      [U, S]
  V    = X @ W_v (natural layout)                       [S, U]
  per 512-wide query chunk:
    for each 128-wide key tile:
      S^T tile = K^T_tile^T . Q^T_chunk  (scores transposed, [k, q])
      E  = exp(S^T * 1/sqrt(U))           (ScalarE, no max-subtraction --
                                           scores are O(1) for this problem)
      O^T  += V_tile^T . E                (PSUM accumulation)  [U, q]
      rsum += ones^T . E                  (row sums, [1, q])
    recipT = 1 / transpose(rsum)          (via K=1 matmul)     [q, 1]
    out = (O^T_slice^T . W_o) * recipT + X + b_o   (proj + deferred softmax
                                                    division + residual)

Matmul operands are bf16 (1 cycle/row on the PE array vs 4 for fp32),
accumulation is fp32 in PSUM.
"""

import sys

if "/opt/trn_rl_repo" not in sys.path:
    sys.path.insert(0, "/opt/trn_rl_repo")

from contextlib import ExitStack

import numpy as np

import concourse.bass as bass
import concourse.tile as tile
from concourse import bacc, mybir
from concourse.bass_utils import run_bass_kernel_spmd
from concourse.masks import make_identity

B, S, D, U, P = 8, 2048, 256, 128, 128
NT = S // P            # 16 key/seq tiles of 128
QC = 512               # query chunk (one PSUM bank of fp32)
NQ = S // QC           # 4 query chunks
SCALE = 1.0 / float(np.sqrt(U))
F32 = mybir.dt.float32
BF16 = mybir.dt.bfloat16
EXP = mybir.ActivationFunctionType.Exp


def build_bass():
    nc = bacc.Bacc("TRN2", target_bir_lowering=False, debug=False)

    x = nc.dram_tensor("inputs", [S, D], F32, kind="ExternalInput").ap()
    wq_d = nc.dram_tensor("W_q", [D, U], F32, kind="ExternalInput").ap()
    wk_d = nc.dram_tensor("W_k", [D, U], F32, kind="ExternalInput").ap()
    wv_d = nc.dram_tensor("W_v", [D, U], F32, kind="ExternalInput").ap()
    wo_d = nc.dram_tensor("W_o", [U, D], F32, kind="ExternalInput").ap()
    bo_d = nc.dram_tensor("b_o", [D], F32, kind="ExternalInput").ap()
    out_d = nc.dram_tensor("out", [S, D], F32, kind="ExternalOutput").ap()

    x_tiled = x.rearrange("(t p) d -> p t d", p=P)
    out_tiled = out_d.rearrange("(t p) d -> p t d", p=P)

    with tile.TileContext(nc) as tc, ExitStack() as ctx:
        consts = ctx.enter_context(tc.tile_pool(name="consts", bufs=1))
        sb = ctx.enter_context(tc.tile_pool(name="sb", bufs=1))
        work = ctx.enter_context(tc.tile_pool(name="work", bufs=4))
        outp = ctx.enter_context(tc.tile_pool(name="outp", bufs=3))
        ps_big = ctx.enter_context(tc.tile_pool(name="ps_big", bufs=3, space="PSUM"))
        ps_sm = ctx.enter_context(tc.tile_pool(name="ps_sm", bufs=2, space="PSUM"))
        ps_acc = ctx.enter_context(tc.tile_pool(name="ps_acc", bufs=2, space="PSUM"))
        ps_rs = ctx.enter_context(tc.tile_pool(name="ps_rs", bufs=1, space="PSUM"))

        # ---- constants ----
        ident_bf = consts.tile([P, P], BF16)
        make_identity(nc, ident_bf)
        ones_bf = consts.tile([P, 1], BF16)
        nc.vector.memset(ones_bf, 1.0)
        ones11_f = consts.tile([1, 1], F32)
        nc.vector.memset(ones11_f, 1.0)
        zbias = consts.tile([P, 1], F32)
        nc.vector.memset(zbias, 0.0)
        bo_bc = consts.tile([P, D], F32)
        bo_bcast_ap = bass.AP(tensor=bo_d.tensor, offset=bo_d.offset,
                              ap=[[0, P]] + list(bo_d.ap))
        nc.sync.dma_start(out=bo_bc[:], in_=bo_bcast_ap)

        def load_w(dram_ap, shape, name):
            f = consts.tile(shape, F32, tag=f"{name}_stage")
            nc.sync.dma_start(out=f[:], in_=dram_ap)
            b = consts.tile(shape, BF16, tag=f"{name}_bf")
            nc.vector.tensor_copy(b[:], f[:])
            return b

        wq_b = load_w(wq_d.rearrange("(c p) u -> p c u", p=P), [P, 2, U], "wq")
        wk_b = load_w(wk_d.rearrange("(c p) u -> p c u", p=P), [P, 2, U], "wk")
        wv_b = load_w(wv_d.rearrange("(c p) u -> p c u", p=P), [P, 2, U], "wv")
        wo_b = load_w(wo_d, [P, D], "wo")

        # ---- X load, residual precompute, X^T, QKV projections ----
        x_nat = sb.tile([P, NT, D], F32)
        x_res = sb.tile([P, NT, D], F32)
        x_bf = sb.tile([P, NT, D], BF16)
        xt_bf = sb.tile([P, 2, S], BF16)  # X^T: [d_part, d_chunk, s]
        for g in range(4):
            sl = slice(4 * g, 4 * (g + 1))
            nc.sync.dma_start(out=x_nat[:, sl, :], in_=x_tiled[:, sl, :])
        for t in range(NT):
            nc.vector.tensor_copy(x_bf[:, t, :], x_nat[:, t, :])
            nc.vector.tensor_add(x_res[:, t, :], x_nat[:, t, :], bo_bc[:])
            for c in range(2):
                xt_ps = ps_sm.tile([P, P], BF16, tag="sm")
                nc.tensor.transpose(xt_ps[:], x_bf[:, t, c * P:(c + 1) * P], ident_bf[:])
                nc.vector.tensor_copy(xt_bf[:, c, t * P:(t + 1) * P], xt_ps[:])

        qt_bf = sb.tile([P, S], BF16)  # Q^T [u, s]
        kt_bf = sb.tile([P, S], BF16)  # K^T [u, s]
        v_bf = sb.tile([P, NT, U], BF16)  # V natural [s_in_tile, t, u]
        for n in range(NQ):
            sl = slice(n * QC, (n + 1) * QC)
            for w_b, dst in ((wq_b, qt_bf), (wk_b, kt_bf)):
                ps = ps_big.tile([P, QC], F32, tag="big")
                nc.tensor.matmul(ps[:], w_b[:, 0, :], xt_bf[:, 0, sl], start=True, stop=False)
                nc.tensor.matmul(ps[:], w_b[:, 1, :], xt_bf[:, 1, sl], start=False, stop=True)
                nc.vector.tensor_copy(dst[:, sl], ps[:])
        for t in range(NT):
            ps = ps_sm.tile([P, P], F32, tag="sm")
            nc.tensor.matmul(ps[:, :U], xt_bf[:, 0, t * P:(t + 1) * P], wv_b[:, 0, :], start=True, stop=False)
            nc.tensor.matmul(ps[:, :U], xt_bf[:, 1, t * P:(t + 1) * P], wv_b[:, 1, :], start=False, stop=True)
            nc.vector.tensor_copy(v_bf[:, t, :], ps[:, :U])

        # ---- attention over query chunks ----
        for qc in range(NQ):
            qsl = slice(qc * QC, (qc + 1) * QC)
            ot_ps = ps_acc.tile([P, QC], F32, tag="acc")  # O^T accum [u, q]
            rs_ps = ps_rs.tile([1, QC], F32, tag="rs")    # row sums [1, q]
            for kt in range(NT):
                sc_ps = ps_big.tile([P, QC], F32, tag="big")
                nc.tensor.matmul(sc_ps[:], kt_bf[:, kt * P:(kt + 1) * P], qt_bf[:, qsl],
                                 start=True, stop=True)
                e_bf = work.tile([P, QC], BF16, tag="exp")
                nc.scalar.activation(e_bf[:], sc_ps[:], EXP, bias=zbias[:], scale=SCALE)
                nc.tensor.matmul(ot_ps[:], v_bf[:, kt, :], e_bf[:],
                                 start=(kt == 0), stop=(kt == NT - 1))
                nc.tensor.matmul(rs_ps[:], ones_bf[:], e_bf[:],
                                 start=(kt == 0), stop=(kt == NT - 1))

            ot_bf = outp.tile([P, QC], BF16, tag="otb")
            nc.vector.tensor_copy(ot_bf[:], ot_ps[:])
            rs_sb = outp.tile([1, QC], F32, tag="rssb")
            nc.scalar.copy(rs_sb[:], rs_ps[:])
            # transpose row sums [1, 512] -> [128, 4] via K=1 matmuls
            rt_ps = ps_sm.tile([P, 4], F32, tag="sm")
            for j in range(4):
                nc.tensor.matmul(rt_ps[:, j:j + 1], rs_sb[:, j * P:(j + 1) * P],
                                 ones11_f[:], start=True, stop=True)
            recip = outp.tile([P, 4], F32, tag="recip")
            nc.vector.reciprocal(recip[:], rt_ps[:])

            for j in range(4):
                qt_i = qc * 4 + j
                pj_ps = ps_big.tile([P, QC], F32, tag="big")
                nc.tensor.matmul(pj_ps[:, :D], ot_bf[:, j * P:(j + 1) * P], wo_b[:],
                                 start=True, stop=True)
                o_sb = outp.tile([P, D], F32, tag="osb")
                nc.vector.scalar_tensor_tensor(
                    o_sb[:], pj_ps[:, :D], recip[:, j:j + 1], x_res[:, qt_i, :],
                    op0=mybir.AluOpType.mult, op1=mybir.AluOpType.add)
                nc.sync.dma_start(out=out_tiled[:, qt_i, :], in_=o_sb[:])

    nc.compile()
    return nc


_NC_CACHE = None


def _get_nc():
    global _NC_CACHE
    if _NC_CACHE is None:
        _NC_CACHE = build_bass()
    return _NC_CACHE


def make_in_maps(inputs, W_q, W_k, W_v, W_o, b_o):
    return [
        {
            "inputs": np.ascontiguousarray(inputs[i], dtype=np.float32),
            "W_q": np.asarray(W_q, dtype=np.float32),
            "W_k": np.asarray(W_k, dtype=np.float32),
            "W_v": np.asarray(W_v, dtype=np.float32),
            "W_o": np.asarray(W_o, dtype=np.float32),
            "b_o": np.asarray(b_o, dtype=np.float32),
        }
        for i in range(B)
    ]


def run_sharded(in_maps, trace=False, **kw):
    nc = _get_nc()
    return run_bass_kernel_spmd(nc, in_maps, core_ids=list(range(B)), trace=trace, **kw)


def kernel(inputs, W_q, W_k, W_v, W_o, b_o):
    inputs = np.asarray(inputs)
    res = run_sharded(make_in_maps(inputs, W_q, W_k, W_v, W_o, b_o))
    out = np.stack([np.asarray(res.results[i]["out"]) for i in range(B)], axis=0)
    return out.astype(np.float32)


if __name__ == "__main__":
    rng = np.random.default_rng(0)
    ins = {
        "inputs": rng.standard_normal((B, S, D), dtype=np.float32),
        "W_q": rng.standard_normal((D, U), dtype=np.float32) / 16.0,
        "W_k": rng.standard_normal((D, U), dtype=np.float32) / 16.0,
        "W_v": rng.standard_normal((D, U), dtype=np.float32) / 16.0,
        "W_o": rng.standard_normal((U, D), dtype=np.float32) / np.sqrt(128.0),
        "b_o": np.zeros((D,), dtype=np.float32),
    }
    out = kernel(**ins)
    print("out", out.shape, out.dtype, float(np.abs(out).mean()))
